# revision 17
# baseline (speedup 1.0000x reference)
"""Trainium2 Bass kernel for the segmented block-diagonal linear layer.

out[b, (seg, v, i)] = sum_u x[b, (seg, u, i)] * W_seg[u, v] / sqrt(mu_seg)

Segments (mul_in, mul_out, ir_dim): (256,256,1) (128,128,3) (64,64,5) (32,32,7)
x: [100000, 1184] f32, weight: [1, 87040] f32 -> out: [100000, 1184] f32

Strategy: data-parallel over 8 NeuronCores (12500 rows each). Host-side, x's
feature columns are permuted to ir-major order per segment ((u,i) -> (i,u))
and cast to fp16 before upload; the output comes back ir-major fp16 and is
un-permuted / upcast on the host. This does two things:

 - ir-major feature pieces make every per-piece weight matrix block-diagonal
   per irrep index, so the PE streams exactly 1440 weight columns per 128-row
   slot (the delta-interleaved mul-major layout needed 3136).
 - fp16 on both HBM streams halves the memory traffic of the fp32 baseline
   (the kernel is HBM-bandwidth-bound at ~312 GB/s/core): 29.6 MB in +
   29.6 MB out per core. fp16 keeps ~2^-11 relative rounding, far inside the
   tolerance, and matches the PE's fp16 compute precision.

Per core, row tiles pack M_PACK consecutive batch rows per SBUF partition so
every DMA descriptor covers M_PACK contiguous DRAM rows. Per 128-row slot:
PE-transpose ten 128-wide feature pieces (via PSUM staging), then one matmul
per piece against host-prepared block-diagonal fp16 weights; outputs land in
PSUM in final (ir-major) feature order and are cast-copied to fp16 SBUF by
the scalar/vector engines. Input DMA on the SP HWDGE ring, output DMA on the
Activation HWDGE ring.
"""

import sys

if "/opt/trn_rl_repo" not in sys.path:
    sys.path.insert(0, "/opt/trn_rl_repo")

import numpy as np

import concourse.bacc as bacc
import concourse.mybir as mybir
from concourse import masks, tile
from concourse.bass_utils import run_bass_kernel_spmd

SEGS = [(256, 256, 1), (128, 128, 3), (64, 64, 5), (32, 32, 7)]
IN_DIM = 1184
N_CORES = 8
M_PACK = 4  # batch rows packed per SBUF partition in the main tiles

# Transpose pieces in the ir-major layout: contiguous chunks (feat_lo, width).
# Piece p is staged at xT columns [128*p, 128*p + width), partitions [0, width).
PIECES = [
    (0, 128), (128, 128),                      # seg0 (256 feats, d=1)
    (256, 128), (384, 128), (512, 128),        # seg1 (384 feats, d=3: i-blocks)
    (640, 128), (768, 128), (896, 64),         # seg2 (320 feats, d=5)
    (960, 128), (1088, 96),                    # seg3 (224 feats, d=7)
]

# Per-piece matmul plan: (psum_bank, psum_col_lo, n_cols, start, stop).
# seg0's two pieces accumulate into the same psum columns; every other piece
# is a self-contained block-diagonal product. seg3 shares bank b0 with seg0.
PIECE_PLAN = [
    ("b0", 0, 256, True, False),   # seg0 u 0:128
    ("b0", 0, 256, False, True),   # seg0 u 128:256
    ("b1", 0, 128, True, True),    # seg1 i=0
    ("b1", 128, 128, True, True),  # seg1 i=1
    ("b1", 256, 128, True, True),  # seg1 i=2
    ("b2", 0, 128, True, True),    # seg2 i=0,1
    ("b2", 128, 128, True, True),  # seg2 i=2,3
    ("b2", 256, 64, True, True),   # seg2 i=4
    ("b0", 256, 128, True, True),  # seg3 i=0..3
    ("b0", 384, 96, True, True),   # seg3 i=4..6
]

# PSUM bank -> (yt feature lo, width, copy engine) for the output copies.
COPY_PLAN = [
    ("b0", 0, 256, 0, "act"),      # seg0
    ("b1", 0, 384, 256, "act"),    # seg1
    ("b2", 0, 320, 640, "vec"),    # seg2
    ("b0", 256, 224, 960, "act"),  # seg3
]

_BUILD_CACHE = {}


def _feature_perm():
    """Logical (mul-major) feature index for each ir-major device column:
    device column off + i*mu + u  <->  logical column off + u*d + i."""
    perm = np.empty(IN_DIM, dtype=np.int64)
    off = 0
    for mu, _mv, d in SEGS:
        idx = np.arange(mu * d).reshape(mu, d).T.reshape(-1)  # (i, u) order
        perm[off : off + mu * d] = off + idx
        off += mu * d
    return perm


_PERM = _feature_perm()


def _prepare_weights(weight):
    """Host-side: per-piece fp16 weight chunks matching the ir-major pieces.
    Piece rows are (i-block, u) features; columns are (i-block, v) outputs, so
    each chunk is block-diagonal with copies of the segment's W / sqrt(mu)."""
    w = np.asarray(weight, dtype=np.float32).reshape(-1)
    Ws = []
    off = 0
    for mu, mv, _d in SEGS:
        Ws.append(w[off : off + mu * mv].reshape(mu, mv) * np.float32(1.0 / np.sqrt(mu)))
        off += mu * mv

    def bd(W, k):
        m, n = W.shape
        D = np.zeros((m * k, n * k), dtype=np.float32)
        for j in range(k):
            D[j * m : (j + 1) * m, j * n : (j + 1) * n] = W
        return D

    chunks = [
        Ws[0][0:128, :],          # p0
        Ws[0][128:256, :],        # p1
        Ws[1], Ws[1], Ws[1],      # p2-4 (seg1 per-i)
        bd(Ws[2], 2), bd(Ws[2], 2), Ws[2],  # p5-7 (seg2: i pairs + i4)
        bd(Ws[3], 4), bd(Ws[3], 3),         # p8-9 (seg3: i0-3, i4-6)
    ]
    return [np.ascontiguousarray(c).astype(np.float16) for c in chunks]


def _build(rows_per_core, w_shapes):
    key = (rows_per_core, tuple(w_shapes))
    if key in _BUILD_CACHE:
        return _BUILD_CACHE[key]

    f32 = mybir.dt.float32
    f16 = mybir.dt.float16

    nc = bacc.Bacc("TRN2", target_bir_lowering=False, debug=False)
    x_d = nc.declare_dram_parameter("x", [rows_per_core, IN_DIM], f16, isOutput=False)
    w_d = [
        nc.declare_dram_parameter(f"wd{i}", list(s), f16, isOutput=False)
        for i, s in enumerate(w_shapes)
    ]
    y_d = nc.declare_dram_parameter("y", [rows_per_core, IN_DIM], f16, isOutput=True)

    # Every tile packs M_PACK consecutive rows per partition: partition p of
    # tile at r0 holds rows r0 + M_PACK*p .. + M_PACK-1. Tiles differ only in
    # partition count (128 for full tiles, fewer for the tail); a sub-M_PACK
    # remainder gets a final 1-row-per-partition tile.
    assert rows_per_core >= M_PACK
    packs, last_rows = divmod(rows_per_core, M_PACK)
    full_tiles = []  # (nparts, m) per tile
    while packs >= 128:
        full_tiles.append((128, M_PACK))
        packs -= 128
    rem = packs * M_PACK + last_rows
    tail_tiles = []
    while rem > 0:
        t = min(rem, 128)
        tail_tiles.append((t, 1))
        rem -= t
    # One tail tile goes FIRST (its small input DMA lands quickly and fills
    # the pipeline ramp); one goes LAST so the kernel drains on a short
    # 1-slot chain instead of a full 4-slot tile.
    tiles = tail_tiles[:1] + full_tiles + tail_tiles[1:]

    with tile.TileContext(nc) as tc:
        with (
            tc.tile_pool(name="wpool", bufs=1) as wpool,
            tc.tile_pool(name="xpool", bufs=3) as xpool,
            tc.tile_pool(name="xtpool", bufs=3) as xtpool,
            tc.tile_pool(name="ypool", bufs=3) as ypool,
            tc.tile_pool(name="stagp", bufs=2, space="PSUM") as stagp,
            tc.tile_pool(name="outp", bufs=2, space="PSUM") as outp,
        ):
            ident = wpool.tile([128, 128], f16)
            masks.make_identity(nc, ident[:])
            # Weight loads go on the Activation HWDGE ring so they don't
            # delay the first x-tile DMA issue on the SP ring.
            wts = []
            for i, s in enumerate(w_shapes):
                wt = wpool.tile(list(s), f16, name=f"wsb{i}")
                nc.scalar.dma_start(out=wt[:], in_=w_d[i][:, :])
                wts.append(wt)

            GROUPS = [0, 4, 8]  # transpose-piece group starts

            def emit_tgroup(slot, g0):
                """Transpose one group of pieces into PSUM staging, then
                DVE-copy into the slot's xT tile."""
                xt, _, xT, _, j, rows = slot
                group = PIECES[g0 : g0 + 4]
                stag = stagp.tile([128, 512], f16, name="stag")
                for k, (flo, width) in enumerate(group):
                    nc.tensor.transpose(
                        stag[:width, k * 128 : k * 128 + rows],
                        xt[:rows, j * IN_DIM + flo : j * IN_DIM + flo + width],
                        ident[:rows, :rows],
                    )
                ncols = len(group) * 128
                nc.vector.tensor_copy(
                    xT[:, g0 * 128 : g0 * 128 + ncols], stag[:, :ncols]
                )

            def emit_mms(slot, pieces):
                _, _, xT, pb, _, rows = slot
                for p in pieces:
                    width = PIECES[p][1]
                    bank, clo, n, start, stop = PIECE_PLAN[p]
                    nc.tensor.matmul(
                        pb[bank][:rows, clo : clo + n],
                        xT[:width, p * 128 : p * 128 + rows],
                        wts[p][:width, :n],
                        start=start,
                        stop=stop,
                    )

            def emit_copies(slot):
                _, yt, _, pb, j, rows = slot
                for bank, clo, fw, flo, eng in COPY_PLAN:
                    src = pb[bank][:rows, clo : clo + fw]
                    dst = yt[:rows, j * IN_DIM + flo : j * IN_DIM + flo + fw]
                    if eng == "act":
                        nc.scalar.copy(out=dst, in_=src)
                    else:
                        nc.vector.tensor_copy(dst, src)

            # Software-pipeline the row-slots: interleave slot s+1's
            # transposes with slot s's matmuls so the PE never sees a long
            # matmul-free window (HAM would re-throttle the clock after
            # ~3.4us without matmul activity).
            pending = None  # slot whose matmuls have not been emitted yet
            MM_GROUPS = [(0, 1, 2, 3), (4, 5, 6, 7), (8, 9)]

            def start_slot(xt, yt, j, rows, finishes_tile):
                nonlocal pending
                xT = xtpool.tile([128, 128 * len(PIECES)], f16, name="xT")
                pb = {
                    "b0": outp.tile([128, 512], f32, name="pb0"),
                    "b1": outp.tile([128, 384], f32, name="pb1"),
                    "b2": outp.tile([128, 320], f32, name="pb2"),
                }
                slot = (xt, yt, xT, pb, j, rows)
                if pending is None:
                    for g in GROUPS:
                        emit_tgroup(slot, g)
                else:
                    prev = pending[0]
                    emit_tgroup(slot, GROUPS[0])
                    emit_mms(prev, MM_GROUPS[0])
                    emit_tgroup(slot, GROUPS[1])
                    emit_mms(prev, MM_GROUPS[1])
                    emit_tgroup(slot, GROUPS[2])
                    emit_mms(prev, MM_GROUPS[2])
                    finish_pending()
                pending = (slot, finishes_tile)

            def finish_pending(emit_all_mms=False):
                nonlocal pending
                if pending is None:
                    return
                slot, fin = pending
                if emit_all_mms:
                    for g in MM_GROUPS:
                        emit_mms(slot, g)
                emit_copies(slot)
                if fin is not None:
                    fin()
                pending = None

            def mk_fin(yt, r0, nparts, m, j0, j1):
                # Output DMA for row-slots [j0, j1) of the tile at r0: m=8
                # tiles store in halves so the pipeline drains earlier.
                def fin():
                    dst = y_d[r0 : r0 + nparts * m, :].rearrange(
                        "(p m) f -> p (m f)", m=m
                    )[:, j0 * IN_DIM : j1 * IN_DIM]
                    nc.scalar.dma_start(
                        out=dst, in_=yt[:nparts, j0 * IN_DIM : j1 * IN_DIM]
                    )

                return fin

            last_full = max(
                (i for i, (np_, m_) in enumerate(tiles) if np_ == 128 and m_ == M_PACK),
                default=-1,
            )
            r0 = 0
            for ti, (nparts, m) in enumerate(tiles):
                trows = nparts * m
                xt = xpool.tile([128, M_PACK * IN_DIM], f16, name="xt")
                src = x_d[r0 : r0 + trows, :].rearrange("(p m) f -> p (m f)", m=m)
                nc.sync.dma_start(out=xt[:nparts, : m * IN_DIM], in_=src)
                yt = ypool.tile([128, M_PACK * IN_DIM], f16, name="yt")

                # The last full tile stores its output in halves so the drain
                # overlaps its own tail compute.
                if m == M_PACK and ti == last_full:
                    halves = [(0, m // 2), (m // 2, m)]
                else:
                    halves = [(0, m)]
                for j0, j1 in halves:
                    for j in range(j0, j1):
                        fin = mk_fin(yt, r0, nparts, m, j0, j1) if j == j1 - 1 else None
                        start_slot(xt, yt, j, nparts, fin)
                r0 += trows

            finish_pending(emit_all_mms=True)

    nc.compile()
    _BUILD_CACHE[key] = nc
    return nc


def _run(x, weight, trace=False, trace_kwargs=None):
    x = np.asarray(x)
    batch = x.shape[0]
    assert batch % N_CORES == 0, f"batch {batch} not divisible by {N_CORES}"
    rows_per_core = batch // N_CORES

    # ir-major permute + fp16 cast for upload (see module docstring).
    x_dev = np.ascontiguousarray(x[:, _PERM], dtype=np.float16)

    wchunks = _prepare_weights(weight)
    nc = _build(rows_per_core, [c.shape for c in wchunks])

    in_maps = []
    for c in range(N_CORES):
        m = {"x": x_dev[c * rows_per_core : (c + 1) * rows_per_core]}
        for i, wc in enumerate(wchunks):
            m[f"wd{i}"] = wc
        in_maps.append(m)

    kwargs = {}
    if trace:
        kwargs["trace"] = True
        if trace_kwargs:
            kwargs["trace_kwargs"] = trace_kwargs
    res = run_bass_kernel_spmd(nc, in_maps, list(range(N_CORES)), **kwargs)
    y_dev = np.concatenate([res.results[c]["y"] for c in range(N_CORES)], axis=0)
    out = np.empty((batch, IN_DIM), dtype=np.float32)
    out[:, _PERM] = y_dev.astype(np.float32)
    return out, res


def kernel(x, weight):
    out, _ = _run(x, weight)
    return out


# revision 19
# speedup vs baseline: 1.0057x; 1.0057x over previous
"""Trainium2 Bass kernel for the segmented block-diagonal linear layer.

out[b, (seg, v, i)] = sum_u x[b, (seg, u, i)] * W_seg[u, v] / sqrt(mu_seg)

Segments (mul_in, mul_out, ir_dim): (256,256,1) (128,128,3) (64,64,5) (32,32,7)
x: [100000, 1184] f32, weight: [1, 87040] f32 -> out: [100000, 1184] f32

Strategy: data-parallel over 8 NeuronCores (12500 rows each). Host-side, x's
feature columns are permuted to ir-major order per segment ((u,i) -> (i,u))
and cast to fp16 before upload; the output comes back ir-major fp16 and is
un-permuted / upcast on the host. This does two things:

 - ir-major feature pieces make every per-piece weight matrix block-diagonal
   per irrep index, so the PE streams exactly 1440 weight columns per 128-row
   slot (the delta-interleaved mul-major layout needed 3136).
 - fp16 on both HBM streams halves the memory traffic of the fp32 baseline
   (the kernel is HBM-bandwidth-bound at ~312 GB/s/core): 29.6 MB in +
   29.6 MB out per core. fp16 keeps ~2^-11 relative rounding, far inside the
   tolerance, and matches the PE's fp16 compute precision.

Per core, row tiles pack M_PACK consecutive batch rows per SBUF partition so
every DMA descriptor covers M_PACK contiguous DRAM rows. Per 128-row slot:
PE-transpose ten 128-wide feature pieces (via PSUM staging), then one matmul
per piece against host-prepared block-diagonal fp16 weights; outputs land in
PSUM in final (ir-major) feature order and are cast-copied to fp16 SBUF by
the scalar/vector engines. Input DMA on the SP HWDGE ring, output DMA on the
Activation HWDGE ring.
"""

import sys

if "/opt/trn_rl_repo" not in sys.path:
    sys.path.insert(0, "/opt/trn_rl_repo")

import numpy as np

import concourse.bacc as bacc
import concourse.mybir as mybir
from concourse import masks, tile
from concourse.bass_utils import run_bass_kernel_spmd

SEGS = [(256, 256, 1), (128, 128, 3), (64, 64, 5), (32, 32, 7)]
IN_DIM = 1184
N_CORES = 8
M_PACK = 4  # batch rows packed per SBUF partition in the main tiles

# Transpose pieces in the ir-major layout: contiguous chunks (feat_lo, width).
# Piece p is staged at xT columns [128*p, 128*p + width), partitions [0, width).
PIECES = [
    (0, 128), (128, 128),                      # seg0 (256 feats, d=1)
    (256, 128), (384, 128), (512, 128),        # seg1 (384 feats, d=3: i-blocks)
    (640, 128), (768, 128), (896, 64),         # seg2 (320 feats, d=5)
    (960, 128), (1088, 96),                    # seg3 (224 feats, d=7)
]

# Per-piece matmul plan: (psum_bank, psum_col_lo, n_cols, start, stop).
# seg0's two pieces accumulate into the same psum columns; every other piece
# is a self-contained block-diagonal product. seg3 shares bank b0 with seg0.
PIECE_PLAN = [
    ("b0", 0, 256, True, False),   # seg0 u 0:128
    ("b0", 0, 256, False, True),   # seg0 u 128:256
    ("b1", 0, 128, True, True),    # seg1 i=0
    ("b1", 128, 128, True, True),  # seg1 i=1
    ("b1", 256, 128, True, True),  # seg1 i=2
    ("b2", 0, 128, True, True),    # seg2 i=0,1
    ("b2", 128, 128, True, True),  # seg2 i=2,3
    ("b2", 256, 64, True, True),   # seg2 i=4
    ("b0", 256, 128, True, True),  # seg3 i=0..3
    ("b0", 384, 96, True, True),   # seg3 i=4..6
]

# PSUM bank -> (yt feature lo, width, copy engine) for the output copies.
COPY_PLAN = [
    ("b0", 0, 256, 0, "act"),      # seg0
    ("b1", 0, 384, 256, "act"),    # seg1
    ("b2", 0, 320, 640, "vec"),    # seg2
    ("b0", 256, 224, 960, "act"),  # seg3
]

_BUILD_CACHE = {}


def _feature_perm():
    """Logical (mul-major) feature index for each ir-major device column:
    device column off + i*mu + u  <->  logical column off + u*d + i."""
    perm = np.empty(IN_DIM, dtype=np.int64)
    off = 0
    for mu, _mv, d in SEGS:
        idx = np.arange(mu * d).reshape(mu, d).T.reshape(-1)  # (i, u) order
        perm[off : off + mu * d] = off + idx
        off += mu * d
    return perm


_PERM = _feature_perm()


def _prepare_weights(weight):
    """Host-side: per-piece fp16 weight chunks matching the ir-major pieces.
    Piece rows are (i-block, u) features; columns are (i-block, v) outputs, so
    each chunk is block-diagonal with copies of the segment's W / sqrt(mu)."""
    w = np.asarray(weight, dtype=np.float32).reshape(-1)
    Ws = []
    off = 0
    for mu, mv, _d in SEGS:
        Ws.append(w[off : off + mu * mv].reshape(mu, mv) * np.float32(1.0 / np.sqrt(mu)))
        off += mu * mv

    def bd(W, k):
        m, n = W.shape
        D = np.zeros((m * k, n * k), dtype=np.float32)
        for j in range(k):
            D[j * m : (j + 1) * m, j * n : (j + 1) * n] = W
        return D

    chunks = [
        Ws[0][0:128, :],          # p0
        Ws[0][128:256, :],        # p1
        Ws[1], Ws[1], Ws[1],      # p2-4 (seg1 per-i)
        bd(Ws[2], 2), bd(Ws[2], 2), Ws[2],  # p5-7 (seg2: i pairs + i4)
        bd(Ws[3], 4), bd(Ws[3], 3),         # p8-9 (seg3: i0-3, i4-6)
    ]
    return [np.ascontiguousarray(c).astype(np.float16) for c in chunks]


def _build(rows_per_core, w_shapes):
    key = (rows_per_core, tuple(w_shapes))
    if key in _BUILD_CACHE:
        return _BUILD_CACHE[key]

    f32 = mybir.dt.float32
    f16 = mybir.dt.float16

    nc = bacc.Bacc("TRN2", target_bir_lowering=False, debug=False)
    x_d = nc.declare_dram_parameter("x", [rows_per_core, IN_DIM], f16, isOutput=False)
    w_d = [
        nc.declare_dram_parameter(f"wd{i}", list(s), f16, isOutput=False)
        for i, s in enumerate(w_shapes)
    ]
    y_d = nc.declare_dram_parameter("y", [rows_per_core, IN_DIM], f16, isOutput=True)

    # Every tile packs M_PACK consecutive rows per partition: partition p of
    # tile at r0 holds rows r0 + M_PACK*p .. + M_PACK-1. Tiles differ only in
    # partition count (128 for full tiles, fewer for the tail); a sub-M_PACK
    # remainder gets a final 1-row-per-partition tile.
    assert rows_per_core >= M_PACK
    packs, last_rows = divmod(rows_per_core, M_PACK)
    full_tiles = []  # (nparts, m) per tile
    while packs >= 128:
        full_tiles.append((128, M_PACK))
        packs -= 128
    rem = packs * M_PACK + last_rows
    tail_tiles = []
    while rem > 0:
        t = min(rem, 128)
        tail_tiles.append((t, 1))
        rem -= t
    # Tail tiles go FIRST: their small input DMAs land quickly and fill the
    # pipeline ramp instead of serializing into the drain at the end.
    tiles = tail_tiles + full_tiles

    with tile.TileContext(nc) as tc:
        with (
            tc.tile_pool(name="wpool", bufs=1) as wpool,
            tc.tile_pool(name="xpool", bufs=3) as xpool,
            tc.tile_pool(name="xtpool", bufs=3) as xtpool,
            tc.tile_pool(name="ypool", bufs=3) as ypool,
            tc.tile_pool(name="stagp", bufs=2, space="PSUM") as stagp,
            tc.tile_pool(name="outp", bufs=2, space="PSUM") as outp,
        ):
            ident = wpool.tile([128, 128], f16)
            masks.make_identity(nc, ident[:])
            # Weight loads go on the Activation HWDGE ring so they don't
            # delay the first x-tile DMA issue on the SP ring.
            wts = []
            for i, s in enumerate(w_shapes):
                wt = wpool.tile(list(s), f16, name=f"wsb{i}")
                nc.scalar.dma_start(out=wt[:], in_=w_d[i][:, :])
                wts.append(wt)

            GROUPS = [0, 4, 8]  # transpose-piece group starts

            def emit_tgroup(slot, g0):
                """Transpose one group of pieces into PSUM staging, then
                DVE-copy into the slot's xT tile."""
                xt, _, xT, _, j, rows = slot
                group = PIECES[g0 : g0 + 4]
                stag = stagp.tile([128, 512], f16, name="stag")
                for k, (flo, width) in enumerate(group):
                    nc.tensor.transpose(
                        stag[:width, k * 128 : k * 128 + rows],
                        xt[:rows, j * IN_DIM + flo : j * IN_DIM + flo + width],
                        ident[:rows, :rows],
                    )
                ncols = len(group) * 128
                nc.vector.tensor_copy(
                    xT[:, g0 * 128 : g0 * 128 + ncols], stag[:, :ncols]
                )

            def emit_mms(slot, pieces):
                _, _, xT, pb, _, rows = slot
                for p in pieces:
                    width = PIECES[p][1]
                    bank, clo, n, start, stop = PIECE_PLAN[p]
                    nc.tensor.matmul(
                        pb[bank][:rows, clo : clo + n],
                        xT[:width, p * 128 : p * 128 + rows],
                        wts[p][:width, :n],
                        start=start,
                        stop=stop,
                    )

            def emit_copies(slot):
                _, yt, _, pb, j, rows = slot
                for bank, clo, fw, flo, eng in COPY_PLAN:
                    src = pb[bank][:rows, clo : clo + fw]
                    dst = yt[:rows, j * IN_DIM + flo : j * IN_DIM + flo + fw]
                    if eng == "act":
                        nc.scalar.copy(out=dst, in_=src)
                    else:
                        nc.vector.tensor_copy(dst, src)

            # Software-pipeline the row-slots: interleave slot s+1's
            # transposes with slot s's matmuls so the PE never sees a long
            # matmul-free window (HAM would re-throttle the clock after
            # ~3.4us without matmul activity).
            pending = None  # slot whose matmuls have not been emitted yet
            MM_GROUPS = [(0, 1, 2, 3), (4, 5, 6, 7), (8, 9)]

            def start_slot(xt, yt, j, rows, finishes_tile):
                nonlocal pending
                xT = xtpool.tile([128, 128 * len(PIECES)], f16, name="xT")
                pb = {
                    "b0": outp.tile([128, 512], f32, name="pb0"),
                    "b1": outp.tile([128, 384], f32, name="pb1"),
                    "b2": outp.tile([128, 320], f32, name="pb2"),
                }
                slot = (xt, yt, xT, pb, j, rows)
                if pending is None:
                    for g in GROUPS:
                        emit_tgroup(slot, g)
                else:
                    prev = pending[0]
                    emit_tgroup(slot, GROUPS[0])
                    emit_mms(prev, MM_GROUPS[0])
                    emit_tgroup(slot, GROUPS[1])
                    emit_mms(prev, MM_GROUPS[1])
                    emit_tgroup(slot, GROUPS[2])
                    emit_mms(prev, MM_GROUPS[2])
                    finish_pending()
                pending = (slot, finishes_tile)

            def finish_pending(emit_all_mms=False):
                nonlocal pending
                if pending is None:
                    return
                slot, fin = pending
                if emit_all_mms:
                    for g in MM_GROUPS:
                        emit_mms(slot, g)
                emit_copies(slot)
                if fin is not None:
                    fin()
                pending = None

            def mk_fin(yt, r0, nparts, m, j0, j1):
                # Output DMA for row-slots [j0, j1) of the tile at r0: m=8
                # tiles store in halves so the pipeline drains earlier.
                def fin():
                    dst = y_d[r0 : r0 + nparts * m, :].rearrange(
                        "(p m) f -> p (m f)", m=m
                    )[:, j0 * IN_DIM : j1 * IN_DIM]
                    nc.scalar.dma_start(
                        out=dst, in_=yt[:nparts, j0 * IN_DIM : j1 * IN_DIM]
                    )

                return fin

            r0 = 0
            for nparts, m in tiles:
                trows = nparts * m
                xt = xpool.tile([128, M_PACK * IN_DIM], f16, name="xt")
                src = x_d[r0 : r0 + trows, :].rearrange("(p m) f -> p (m f)", m=m)
                nc.sync.dma_start(out=xt[:nparts, : m * IN_DIM], in_=src)
                yt = ypool.tile([128, M_PACK * IN_DIM], f16, name="yt")

                for j in range(m):
                    fin = mk_fin(yt, r0, nparts, m, 0, m) if j == m - 1 else None
                    start_slot(xt, yt, j, nparts, fin)
                r0 += trows

            finish_pending(emit_all_mms=True)

    nc.compile()
    _BUILD_CACHE[key] = nc
    return nc


def _run(x, weight, trace=False, trace_kwargs=None):
    x = np.asarray(x)
    batch = x.shape[0]
    assert batch % N_CORES == 0, f"batch {batch} not divisible by {N_CORES}"
    rows_per_core = batch // N_CORES

    # ir-major permute + fp16 cast for upload (see module docstring).
    x_dev = np.ascontiguousarray(x[:, _PERM], dtype=np.float16)

    wchunks = _prepare_weights(weight)
    nc = _build(rows_per_core, [c.shape for c in wchunks])

    in_maps = []
    for c in range(N_CORES):
        m = {"x": x_dev[c * rows_per_core : (c + 1) * rows_per_core]}
        for i, wc in enumerate(wchunks):
            m[f"wd{i}"] = wc
        in_maps.append(m)

    kwargs = {}
    if trace:
        kwargs["trace"] = True
        if trace_kwargs:
            kwargs["trace_kwargs"] = trace_kwargs
    res = run_bass_kernel_spmd(nc, in_maps, list(range(N_CORES)), **kwargs)
    y_dev = np.concatenate([res.results[c]["y"] for c in range(N_CORES)], axis=0)
    out = np.empty((batch, IN_DIM), dtype=np.float32)
    out[:, _PERM] = y_dev.astype(np.float32)
    return out, res


def kernel(x, weight):
    out, _ = _run(x, weight)
    return out
